# revision 13
# baseline (speedup 1.0000x reference)
"""Radius neighbor search (nn_NeighborSearch) on 8 Trainium2 cores.

Device: all 144M pairwise squared distances via a K=13 bf16 two-term-split
matmul (exact-in-fp32 products, PSUM accumulate), thresholded at
T2 = radius^2 + margin into an int8 candidate mask (over-inclusive by
construction: measured |sq_dev - sq_fp64| <= 5.9e-5 << margin).

Host: exact recomputation of the reference's XLA-CPU fp32 arithmetic at the
candidate pairs only (bitwise-validated fma-chain emulation), then CSR
assembly. All three outputs are bitwise-identical to the CPU reference.
"""

import numpy as np
import ml_dtypes

_BF16 = ml_dtypes.bfloat16
_F32 = np.float32

N_CORES = 8
M_TOTAL = 12000      # queries
N_DATA = 12000       # data points
M_PER_CORE = M_TOTAL // N_CORES   # 1500
STRIP = 125          # query rows per strip (PSUM partition dim)
N_TILE = 500         # data cols per matmul tile (fits one PSUM bank in fp32)
N_STRIPS = M_PER_CORE // STRIP    # 12
N_CTILES = N_DATA // N_TILE       # 24
N_PAIRS = N_CTILES // 2           # 12 (two matmuls share one cmp)
K_FEAT = 13
MAX_NBRS = 262144
EPS = np.float32(1e-7)
MARGIN = 1e-3        # threshold slack; ~17x the max observed device sq error

_NC_CACHE = {}


def _split(a):
    hi = a.astype(_BF16).astype(_F32)
    lo = (a - hi).astype(_BF16).astype(_F32)
    return hi, lo


def _features(data, queries):
    """Build lhsT query features [13, M] and rhs data features [13, N], bf16.

    sq ~= sum_k qw[k,i] * dx[k,j]:
      rows 0-2: (-2*qh_c) * dh_c     rows 3-5: (-2*qh_c) * dl_c
      rows 6-8: (-2*ql_c) * dh_c     (ql*dl term dropped, ~2^-18)
      row 9/10: q2h/q2l * 1          row 11/12: 1 * d2h/d2l
    """
    q = queries.astype(_F32)
    d = data.astype(_F32)
    q2 = (q[:, 0] * q[:, 0] + q[:, 1] * q[:, 1]) + q[:, 2] * q[:, 2]
    d2 = (d[:, 0] * d[:, 0] + d[:, 1] * d[:, 1]) + d[:, 2] * d[:, 2]
    qh, ql = _split(q)
    dh, dl = _split(d)
    q2h, q2l = _split(q2)
    d2h, d2l = _split(d2)

    qw = np.empty((K_FEAT, q.shape[0]), _F32)
    dx = np.empty((K_FEAT, d.shape[0]), _F32)
    for c in range(3):
        qw[c] = _F32(-2.0) * qh[:, c]
        dx[c] = dh[:, c]
        qw[3 + c] = _F32(-2.0) * qh[:, c]
        dx[3 + c] = dl[:, c]
        qw[6 + c] = _F32(-2.0) * ql[:, c]
        dx[6 + c] = dh[:, c]
    qw[9] = q2h
    qw[10] = q2l
    dx[9] = 1.0
    dx[10] = 1.0
    qw[11] = 1.0
    qw[12] = 1.0
    dx[11] = d2h
    dx[12] = d2l
    return qw.astype(_BF16), dx.astype(_BF16)


def _build_program(t2: float):
    import concourse.bass as bass
    import concourse.tile as tile
    from concourse import bacc, mybir

    nc = bacc.Bacc("TRN2", target_bir_lowering=False, debug=False,
                   num_devices=N_CORES)
    qw_d = nc.dram_tensor("qw", (K_FEAT, M_PER_CORE), mybir.dt.bfloat16,
                          kind="ExternalInput")
    dx_d = nc.dram_tensor("dx", (K_FEAT, N_DATA), mybir.dt.bfloat16,
                          kind="ExternalInput")
    # mask laid out [rows, pair, bank, col] == row-major [1500, 12000]
    mask_d = nc.dram_tensor("mask", (M_PER_CORE, N_PAIRS, 2, N_TILE),
                            mybir.dt.int8, kind="ExternalOutput")

    H = N_PAIRS // 2  # 6 pairs per DMA half
    with tile.TileContext(nc) as tc:
        with (
            tc.tile_pool(name="feat", bufs=1) as feat,
            tc.tile_pool(name="strips", bufs=4) as strips,
            tc.tile_pool(name="psum", bufs=4, space="PSUM") as psum,
        ):
            qw_s = feat.tile((K_FEAT, M_PER_CORE), mybir.dt.bfloat16)
            dx_s = feat.tile((K_FEAT, N_DATA), mybir.dt.bfloat16)
            bias_t = feat.tile((128, 1), mybir.dt.float32)
            nc.gpsimd.memset(bias_t[:], float(t2))
            nc.sync.dma_start(qw_s[:], qw_d[:])
            nc.sync.dma_start(dx_s[:], dx_d[:])

            for s in range(N_STRIPS):
                mh0 = strips.tile((STRIP, 4, 2, N_TILE), mybir.dt.int8)
                mh1 = strips.tile((STRIP, 4, 2, N_TILE), mybir.dt.int8)
                mh2 = strips.tile((STRIP, 4, 2, N_TILE), mybir.dt.int8)
                groups = [(mh0, 0), (mh1, 4), (mh2, 8)]
                for p in range(N_PAIRS):
                    ps = psum.tile((STRIP, 2, 512), mybir.dt.float32)
                    for j in range(2):
                        nc.tensor.matmul(
                            ps[:, j, 0:N_TILE],
                            qw_s[:, bass.ts(s, STRIP)],
                            dx_s[:, bass.ts(2 * p + j, N_TILE)],
                            start=True,
                            stop=True,
                        )
                    mt, base = groups[p // 4]
                    dst = mt[:, p - base]
                    if p % 2 == 0:
                        nc.vector.tensor_scalar(
                            dst, ps[:, :, 0:N_TILE], float(t2), None,
                            mybir.AluOpType.is_le)
                    else:
                        nc.scalar.activation(
                            dst, ps[:, :, 0:N_TILE],
                            mybir.ActivationFunctionType.Sign,
                            bias=bias_t[0:STRIP], scale=-1.0)
                    if p == 3:
                        nc.gpsimd.dma_start(
                            mask_d[bass.ts(s, STRIP), 0:4], mh0[:])
                    elif p == 7:
                        nc.gpsimd.dma_start(
                            mask_d[bass.ts(s, STRIP), 4:8], mh1[:])
                nc.gpsimd.dma_start(
                    mask_d[bass.ts(s, STRIP), 8:12], mh2[:])
    nc.compile()
    return nc


def _get_nc(t2: float):
    key = round(float(t2), 12)
    if key not in _NC_CACHE:
        _NC_CACHE[key] = _build_program(t2)
    return _NC_CACHE[key]


def run_device(data, queries, radius, trace=False):
    """Run the SPMD mask kernel; returns (mask [12000,12000] int8, exec_ns)."""
    from concourse.bass_utils import run_bass_kernel_spmd

    r = np.float32(radius)
    t2 = float(r) * float(r) + MARGIN
    qw, dx = _features(np.asarray(data), np.asarray(queries))
    dx = np.ascontiguousarray(dx)
    in_maps = [
        {"qw": np.ascontiguousarray(qw[:, c * M_PER_CORE:(c + 1) * M_PER_CORE]),
         "dx": dx}
        for c in range(N_CORES)
    ]
    nc = _get_nc(t2)
    res = run_bass_kernel_spmd(nc, in_maps, core_ids=list(range(N_CORES)),
                               trace=trace)
    mask = np.concatenate(
        [np.asarray(res.results[c]["mask"]).reshape(M_PER_CORE, N_DATA)
         for c in range(N_CORES)], axis=0)
    return mask, res.exec_time_ns


def kernel(data, queries, radius):
    data = np.ascontiguousarray(np.asarray(data, _F32))
    queries = np.ascontiguousarray(np.asarray(queries, _F32))
    r = np.float32(radius)

    mask, _ = run_device(data, queries, r)
    rows, cols = np.nonzero(mask == 1)

    # Exact XLA-CPU fp32 emulation at candidate pairs (bitwise-validated):
    # qd via fp32 fma chain in k-order, q2/d2 via plain sequential adds.
    q2 = (queries[:, 0] * queries[:, 0] + queries[:, 1] * queries[:, 1]) \
        + queries[:, 2] * queries[:, 2]
    d2 = (data[:, 0] * data[:, 0] + data[:, 1] * data[:, 1]) \
        + data[:, 2] * data[:, 2]
    qr = queries[rows].astype(np.float64)
    dc = data[cols].astype(np.float64)
    acc = (qr[:, 0] * dc[:, 0]).astype(_F32).astype(np.float64)
    acc = (qr[:, 1] * dc[:, 1] + acc).astype(_F32).astype(np.float64)
    qd = (qr[:, 2] * dc[:, 2] + acc).astype(_F32)
    sq = (q2[rows] + d2[cols]) - _F32(2.0) * qd
    ad = np.sqrt(np.maximum(sq, _F32(0.0)))
    ad = np.where(ad == _F32(0.0), EPS, ad)
    keep = ad <= r

    krows = rows[keep]
    kcols = cols[keep]
    kvals = ad[keep]

    dists = np.zeros((M_TOTAL, N_DATA), _F32)
    dists[krows, kcols] = kvals

    counts = np.bincount(krows, minlength=M_TOTAL)
    splits = np.zeros(M_TOTAL + 1, np.int64)
    splits[1:] = np.cumsum(counts)
    splits = splits.astype(np.int32)

    neighbors_index = np.zeros(MAX_NBRS, np.int32)
    neighbors_index[:kcols.size] = kcols

    return dists, neighbors_index, splits


# revision 19
# speedup vs baseline: 1.0443x; 1.0443x over previous
"""Radius neighbor search (nn_NeighborSearch) on 8 Trainium2 cores.

Device: all 144M pairwise squared distances via a 24-logical-row fp8e4m3
three-term-split matmul in DoubleRow perf mode (fp8 products exact in fp32,
PSUM accumulate), thresholded at T2 = radius^2 + margin into an int8
candidate mask (over-inclusive by construction: simulated
max |sq_dev - sq_f64| = 1.68e-3 << margin=2e-3... margin chosen 2x above the
9.5e-4 minimum; ~168k candidates vs ~71k true pairs).

Host: exact recomputation of the reference's XLA-CPU fp32 arithmetic at the
candidate pairs only (bitwise-validated fma-chain emulation), then CSR
assembly. All three outputs are bitwise-identical to the CPU reference.
"""

import numpy as np
import ml_dtypes

_F8 = ml_dtypes.float8_e4m3
_F32 = np.float32
_F64 = np.float64

N_CORES = 8
M_TOTAL = 12000      # queries
N_DATA = 12000       # data points
M_PER_CORE = M_TOTAL // N_CORES   # 1500
M_PAD_CORE = 1536    # padded to 12 strips x 128 (dual-fp8 LDWEIGHTS needs
                     # even col counts / aligned strip offsets; pad rows are
                     # engineered to sq'=256 > T2 so they never hit the mask)
STRIP = 128          # query rows per strip (PSUM partition dim)
N_TILE = 500         # data cols per matmul tile (fits one PSUM bank in fp32)
N_STRIPS = M_PAD_CORE // STRIP    # 12
N_CTILES = N_DATA // N_TILE       # 24
N_PAIRS = N_CTILES // 2           # 12 (two matmuls share one cmp)
K_PHYS = 12          # PE partition rows; 24 logical rows DoubleRow-paired
MAX_NBRS = 262144
EPS = np.float32(1e-7)
MARGIN = 2e-3

_NC_CACHE = {}


def _f8(x):
    return np.asarray(x, _F64).astype(_F8).astype(_F64)


def _split3(x):
    t0 = _f8(x)
    r1 = x - t0
    t1 = _f8(r1 * 16.0) / 16.0
    r2 = r1 - t1
    t2 = _f8(r2 * 256.0) / 256.0
    return t0, t1, t2


# cross-product terms (i, j, shift): lhs = fp8(-2*q_i*2^s), rhs = fp8(d_j*2^-s)
_XPAIRS = [(0, 0, 0), (1, 0, 2), (0, 1, -2), (1, 1, 0), (2, 0, 4), (0, 2, -4)]


def _features(data, queries):
    """24 logical rows -> qw [12,2,M] fp8 (queries, lhsT), dx [12,2,N] fp8."""
    q = queries.astype(_F64)
    d = data.astype(_F64)
    qs = [_split3(q[:, c]) for c in range(3)]
    dsp = [_split3(d[:, c]) for c in range(3)]
    rows_l, rows_r = [], []
    for c in range(3):
        for i, j, s in _XPAIRS:
            rows_l.append(_f8(-2.0 * qs[c][i] * 2.0 ** s))
            rows_r.append(_f8(dsp[c][j] * 2.0 ** (-s)))
    q2 = (q * q).sum(1)
    d2 = (d * d).sum(1)
    ones_m = np.ones(q.shape[0])
    ones_n = np.ones(d.shape[0])
    t0 = _f8(q2)
    rr = q2 - t0
    t1 = _f8(rr * 16.0)
    rr = rr - t1 / 16.0
    t2 = _f8(rr * 256.0)
    rows_l += [t0, t1, t2]
    rows_r += [ones_n, ones_n * 2.0 ** -4, ones_n * 2.0 ** -8]
    u0 = _f8(d2)
    rr = d2 - u0
    u1 = _f8(rr * 16.0)
    rr = rr - u1 / 16.0
    u2 = _f8(rr * 256.0)
    rows_l += [ones_m, ones_m * 2.0 ** -4, ones_m * 2.0 ** -8]
    rows_r += [u0, u1, u2]
    L = np.stack(rows_l).astype(_F8).reshape(K_PHYS, 2, -1)
    R = np.stack(rows_r).astype(_F8).reshape(K_PHYS, 2, -1)
    return L, R


def _build_program(t2: float):
    import concourse.bass as bass
    import concourse.tile as tile
    from concourse import bacc, mybir

    nc = bacc.Bacc("TRN2", target_bir_lowering=False, debug=False,
                   num_devices=N_CORES)
    qw_d = nc.dram_tensor("qw", (K_PHYS, 2, M_PAD_CORE), mybir.dt.float8e4,
                          kind="ExternalInput")
    dx_d = nc.dram_tensor("dx", (K_PHYS, 2, N_DATA), mybir.dt.float8e4,
                          kind="ExternalInput")
    # mask laid out [rows, pair, bank, col] == row-major [1536, 12000]
    mask_d = nc.dram_tensor("mask", (M_PAD_CORE, N_PAIRS, 2, N_TILE),
                            mybir.dt.int8, kind="ExternalOutput")

    with tile.TileContext(nc) as tc:
        with (
            tc.tile_pool(name="feat", bufs=1) as feat,
            tc.tile_pool(name="strips", bufs=4) as strips,
            tc.tile_pool(name="psum", bufs=4, space="PSUM") as psum,
        ):
            qw_s = feat.tile((K_PHYS, 2, M_PAD_CORE), mybir.dt.float8e4)
            dx_s = feat.tile((K_PHYS, 2, N_DATA), mybir.dt.float8e4)
            bias_t = feat.tile((128, 1), mybir.dt.float32)
            nc.gpsimd.memset(bias_t[:], float(t2))
            nc.sync.dma_start(qw_s[:], qw_d[:])
            nc.sync.dma_start(dx_s[:], dx_d[:])

            for s in range(N_STRIPS):
                mh0 = strips.tile((STRIP, 4, 2, N_TILE), mybir.dt.int8)
                mh1 = strips.tile((STRIP, 4, 2, N_TILE), mybir.dt.int8)
                mh2 = strips.tile((STRIP, 4, 2, N_TILE), mybir.dt.int8)
                groups = [(mh0, 0), (mh1, 4), (mh2, 8)]
                for p in range(N_PAIRS):
                    ps = psum.tile((STRIP, 2, 512), mybir.dt.float32)
                    for j in range(2):
                        nc.tensor.matmul(
                            ps[:, j, 0:N_TILE],
                            qw_s[:, :, bass.ts(s, STRIP)],
                            dx_s[:, :, bass.ts(2 * p + j, N_TILE)],
                            start=True,
                            stop=True,
                            perf_mode=mybir.MatmulPerfMode.DoubleRow,
                        )
                    mt, base = groups[p // 4]
                    dst = mt[:, p - base]
                    if p % 2 == 0:
                        nc.vector.tensor_scalar(
                            dst, ps[:, :, 0:N_TILE], float(t2), None,
                            mybir.AluOpType.is_le)
                    else:
                        nc.scalar.activation(
                            dst, ps[:, :, 0:N_TILE],
                            mybir.ActivationFunctionType.Sign,
                            bias=bias_t[0:STRIP], scale=-1.0)
                    if p == 3:
                        nc.gpsimd.dma_start(
                            mask_d[bass.ts(s, STRIP), 0:4], mh0[:])
                    elif p == 7:
                        nc.gpsimd.dma_start(
                            mask_d[bass.ts(s, STRIP), 4:8], mh1[:])
                nc.gpsimd.dma_start(
                    mask_d[bass.ts(s, STRIP), 8:12], mh2[:])
    nc.compile()
    return nc


def _get_nc(t2: float):
    key = round(float(t2), 12)
    if key not in _NC_CACHE:
        _NC_CACHE[key] = _build_program(t2)
    return _NC_CACHE[key]


def run_device(data, queries, radius, trace=False):
    """Run the SPMD mask kernel; returns (mask [12000,12000] int8, exec_ns)."""
    from concourse.bass_utils import run_bass_kernel_spmd

    r = np.float32(radius)
    t2 = float(r) * float(r) + MARGIN
    qw, dx = _features(np.asarray(data), np.asarray(queries))
    dx = np.ascontiguousarray(dx)
    in_maps = []
    for c in range(N_CORES):
        qwc = np.zeros((K_PHYS, 2, M_PAD_CORE), _F8)
        qwc[:, :, :M_PER_CORE] = \
            qw[:, :, c * M_PER_CORE:(c + 1) * M_PER_CORE]
        # pad rows: q2-t0 row := 256 so sq' = 256 + d2 > t2, mask stays 0
        qwc[9, 0, M_PER_CORE:] = _F8(256.0)
        qwc[10, 1, M_PER_CORE:] = _F8(1.0)
        qwc[11, 0, M_PER_CORE:] = _F8(2.0 ** -4)
        qwc[11, 1, M_PER_CORE:] = _F8(2.0 ** -8)
        in_maps.append({"qw": qwc, "dx": dx})
    nc = _get_nc(t2)
    res = run_bass_kernel_spmd(nc, in_maps, core_ids=list(range(N_CORES)),
                               trace=trace)
    mask = np.concatenate(
        [np.asarray(res.results[c]["mask"]).reshape(M_PAD_CORE, N_DATA)
         [:M_PER_CORE]
         for c in range(N_CORES)], axis=0)
    return mask, res.exec_time_ns


def kernel(data, queries, radius):
    data = np.ascontiguousarray(np.asarray(data, _F32))
    queries = np.ascontiguousarray(np.asarray(queries, _F32))
    r = np.float32(radius)

    mask, _ = run_device(data, queries, r)
    rows, cols = np.nonzero(mask == 1)

    # Exact XLA-CPU fp32 emulation at candidate pairs (bitwise-validated):
    # qd via fp32 fma chain in k-order, q2/d2 via plain sequential adds.
    q2 = (queries[:, 0] * queries[:, 0] + queries[:, 1] * queries[:, 1]) \
        + queries[:, 2] * queries[:, 2]
    d2 = (data[:, 0] * data[:, 0] + data[:, 1] * data[:, 1]) \
        + data[:, 2] * data[:, 2]
    qr = queries[rows].astype(np.float64)
    dc = data[cols].astype(np.float64)
    acc = (qr[:, 0] * dc[:, 0]).astype(_F32).astype(np.float64)
    acc = (qr[:, 1] * dc[:, 1] + acc).astype(_F32).astype(np.float64)
    qd = (qr[:, 2] * dc[:, 2] + acc).astype(_F32)
    sq = (q2[rows] + d2[cols]) - _F32(2.0) * qd
    ad = np.sqrt(np.maximum(sq, _F32(0.0)))
    ad = np.where(ad == _F32(0.0), EPS, ad)
    keep = ad <= r

    krows = rows[keep]
    kcols = cols[keep]
    kvals = ad[keep]

    dists = np.zeros((M_TOTAL, N_DATA), _F32)
    dists[krows, kcols] = kvals

    counts = np.bincount(krows, minlength=M_TOTAL)
    splits = np.zeros(M_TOTAL + 1, np.int64)
    splits[1:] = np.cumsum(counts)
    splits = splits.astype(np.int32)

    neighbors_index = np.zeros(MAX_NBRS, np.int32)
    neighbors_index[:kcols.size] = kcols

    return dists, neighbors_index, splits
